# revision 21
# baseline (speedup 1.0000x reference)
"""CategoricalGCNEncoder on 8 Trainium2 NeuronCores (Bass/Tile).

Design ("v6" — bf16 + engine rebalance over v5):
  - Nodes (dst) sharded across 8 cores; per-layer feature tables AllGathered.
  - Everything on the matmul path is bf16 (fp32 matmuls cost 4x stream +
    2x LDWEIGHTS); PSUM accumulation stays fp32, epilogue math fp32.
  - Slot numbering is p-major (slot = p*W + w) so bounce DMAs are
    per-partition contiguous, issued in window chunks and overlapped with
    compute instead of serializing before each AllGather.
  - Embedding + first matmul fused: h1 = sum_f onehot_f @ T_f with
    T_f = emb_f @ W1_f computed on device; one-hot is uploaded bf16 in a
    window-contiguous layout [NCAT, W, NF, P].
  - GCN normalization folded into node scaling: table rows hold dis*h;
    out[d] = dis[d] * (sum_{e:dst=d} ht[src] + ht[d]) + b.
  - Edge phase: per-edge rows gathered with gpsimd.dma_gather (int16 idx,
    256B bf16-padded rows) from the AllGathered table, 4 SWDGE queues =
    4 src "buckets"; gathers batched 8 windows at a time.
  - Segment-sum: per dst-window (128 nodes) PSUM accumulation of
    matmul(lhsT=S_col[128x128], rhs=msg_col[128xF]) where S is the one-hot
    dst-slot selection matrix generated on DVE via is_equal(iota, dstrel),
    all bf16. Pad edges carry dstrel=-1 -> zero S column.
  - Epilogue per-partition scale/bias ops run on the (otherwise idle)
    Scalar engine via activation(scale=AP,bias=AP); DVE keeps only true
    tensor-tensor work.
  - Host packs nodes into windows (vector bin packing) so every (window,
    bucket) has exactly 4 columns of 128 edge slots; the node->slot
    permutation is undone on the host at the end.
"""

import numpy as np
import ml_dtypes

import concourse.bass as bass
import concourse.mybir as mybir
import concourse.tile as tile
from concourse import bacc
from concourse.bass_utils import run_bass_kernel_spmd

# ---------------- problem constants (hardcoded; kernel must be self-contained)
N = 100000
E = 1600000
NF = 8
EMB = 16
IN_DIM = 128
HID = 64
OUT = 32
NCAT = 100
EPS = 1e-5

NCORE = 8
SH = N // NCORE            # 12500 nodes per core
P = 128
W = 104                    # windows per core
SLOTS = W * P              # 13312 slots per core (>= SH)
KQ = 4                     # columns per (window, bucket)
NQ = 4                     # src buckets == SWDGE queues
COLS = W * KQ              # columns per bucket stream (416)
TOTCOL = NQ * COLS         # total columns (1664)
TOTPOS = TOTCOL * P        # total edge slots (212992)
TBL = NCORE * SLOTS        # table rows (106496)
BUCK = TBL // NQ           # bucket size (26624) < 32768
GW = 13                    # windows per gather group
NG = W // GW               # gather groups (8)
CAP_Q = KQ * P             # 512 edge slots per (w, q)
ROW = 128                  # bf16 elems per table row (= 256B, SWDGE minimum)
CH = 13                    # bounce-DMA chunk (windows)
AGW = 26                   # windows per chunked AllGather (4 chunks)

f32 = mybir.dt.float32
bf16 = mybir.dt.bfloat16
i16 = mybir.dt.int16

_CACHE = {}


# ------------------------------------------------------------------ program
def build_program():
    nc = bacc.Bacc(None, target_bir_lowering=False, debug=False,
                   num_devices=NCORE, num_swdge_queues=NQ,
                   dynamic_dma_scratch_size=16384)
    with tile.TileContext(nc) as tc:
        _build(nc, tc)
    nc.compile()
    return nc


def _build(nc, tc):
    AF = mybir.ActivationFunctionType
    ALU = mybir.AluOpType

    from contextlib import ExitStack
    ctx = ExitStack()
    dram = ctx.enter_context(tc.tile_pool(name="dram", bufs=1, space="DRAM"))
    const = ctx.enter_context(tc.tile_pool(name="const", bufs=1))
    oh_pool = ctx.enter_context(tc.tile_pool(name="ohp", bufs=3))
    msg_pool = ctx.enter_context(tc.tile_pool(name="msgp", bufs=6))
    s_pool = ctx.enter_context(tc.tile_pool(name="sp", bufs=3))
    epi_pool = ctx.enter_context(tc.tile_pool(name="epip", bufs=3))
    psum_mm = ctx.enter_context(tc.tile_pool(name="psmm", bufs=3, space="PSUM"))
    psum_tr = ctx.enter_context(tc.tile_pool(name="pstr", bufs=2, space="PSUM"))
    psum_w2 = ctx.enter_context(tc.tile_pool(name="psw2", bufs=2, space="PSUM"))

    def din(name, shape, dtype=f32):
        return dram.tile(shape, dtype, kind="ExternalInput", name=name,
                         uniquify=False)

    # ---- inputs
    onehot = din("onehot", [NCAT, W, NF, P], bf16)
    idxs = din("idxs", [P, TOTPOS // 16], i16)
    dstrel = din("dstrel", [P, TOTCOL], bf16)
    degin = din("deg", [P, W])
    embT = din("embT", [EMB, NF * NCAT])
    w1 = din("w1", [EMB, NF, HID])
    w2 = din("w2", [HID, OUT], bf16)     # pre-scaled by gamma1 on host
    c2r = din("c2r", [P, OUT])           # beta1 @ W2, replicated
    b1r = din("b1r", [P, HID])
    b2r = din("b2r", [P, OUT])
    g2r = din("g2r", [P, OUT])
    be2r = din("be2r", [P, OUT])
    iotain = din("iota", [P, P], bf16)
    identin = din("ident", [P, P], bf16)

    outx = dram.tile([SLOTS, OUT], f32, kind="ExternalOutput", name="outx",
                     uniquify=False)

    # unpadded bounce (local contribution) + per-chunk AllGather outputs;
    # the padded gather tables are plain DRAM filled by local "repad" DMAs.
    # Gathered 256B rows are [payload, garbage]; the garbage is never read.
    NAG = W // AGW
    # per-chunk contiguous bounce tensors (collective inputs must be
    # contiguous): row (p*AGW + wrel) <-> h1pad[:, a0+wrel, :]
    bounce1 = [dram.tile([P, AGW, HID], bf16, name=f"bounce1_{c}")
               for c in range(NAG)]
    bounce2 = [dram.tile([P, AGW, OUT], bf16, name=f"bounce2_{c}")
               for c in range(NAG)]
    agt1 = [dram.tile([NCORE, P, AGW, HID], bf16, addr_space="Shared",
                      name=f"agt1_{c}") for c in range(NAG)]
    agt2 = [dram.tile([NCORE, P, AGW, OUT], bf16, addr_space="Shared",
                      name=f"agt2_{c}") for c in range(NAG)]
    table1 = dram.tile([TBL, ROW], bf16)
    table2 = dram.tile([TBL, ROW], bf16)
    # table row (k*SLOTS + p*W + w) <-> [k, p, w, :] for repad writes
    table1_v = table1.rearrange("(k p w) h -> k p w h", k=NCORE, p=P)
    table2_v = table2.rearrange("(k p w) h -> k p w h", k=NCORE, p=P)

    def ag_chunk(c, bounce, agt, table_v, fdim):
        """AllGather chunk c of the bounce, then repad into the gather table."""
        a0 = c * AGW
        nc.gpsimd.collective_compute(
            "AllGather", mybir.AluOpType.bypass,
            replica_groups=[list(range(NCORE))],
            ins=[bounce[c][:]], outs=[agt[c][:]],
        )
        for k in range(NCORE):
            nc.sync.dma_start(out=table_v[k, :, a0:a0 + AGW, :fdim],
                              in_=agt[c][k])

    # ---- static SBUF
    idx_sb = const.tile([P, TOTPOS // 16], i16)
    nc.sync.dma_start(out=idx_sb[:], in_=idxs[:])
    dstrel_sb = const.tile([P, TOTCOL], bf16)
    nc.sync.dma_start(out=dstrel_sb[:], in_=dstrel[:])
    iota_sb = const.tile([P, P], bf16)
    nc.sync.dma_start(out=iota_sb[:], in_=iotain[:])
    ident_sb = const.tile([P, P], bf16)
    nc.sync.dma_start(out=ident_sb[:], in_=identin[:])
    w1_sb = const.tile([EMB, NF, HID], f32)
    nc.sync.dma_start(out=w1_sb[:], in_=w1[:])
    w2_sb = const.tile([HID, OUT], bf16)
    nc.sync.dma_start(out=w2_sb[:], in_=w2[:])
    embT_sb = const.tile([EMB, NF * NCAT], f32)
    nc.sync.dma_start(out=embT_sb[:], in_=embT[:])
    b1_sb = const.tile([P, HID], f32)
    nc.sync.dma_start(out=b1_sb[:], in_=b1r[:])
    c2_sb = const.tile([P, OUT], f32)
    nc.sync.dma_start(out=c2_sb[:], in_=c2r[:])
    b2_sb = const.tile([P, OUT], f32)
    nc.sync.dma_start(out=b2_sb[:], in_=b2r[:])
    g2_sb = const.tile([P, OUT], f32)
    nc.sync.dma_start(out=g2_sb[:], in_=g2r[:])
    be2_sb = const.tile([P, OUT], f32)
    nc.sync.dma_start(out=be2_sb[:], in_=be2r[:])
    eps_sb = const.tile([P, 1], f32)
    nc.vector.memset(eps_sb[:], EPS)

    # dis = 1/sqrt(deg)
    deg_sb = const.tile([P, W], f32)
    nc.sync.dma_start(out=deg_sb[:], in_=degin[:])
    dis_sb = const.tile([P, W], f32)
    nc.scalar.activation(out=dis_sb[:], in_=deg_sb[:], func=AF.Sqrt)
    nc.vector.reciprocal(out=dis_sb[:], in_=dis_sb[:])

    # ---- T_f = emb_f @ W1_f  -> T_sb [NCAT, NF, HID] bf16
    T_sb = const.tile([NCAT, NF, HID], bf16)
    for f in range(NF):
        pt = psum_mm.tile([NCAT, HID], f32, space="PSUM", tag="ps")
        nc.tensor.matmul(
            out=pt[:],
            lhsT=embT_sb[:, f * NCAT:(f + 1) * NCAT],
            rhs=w1_sb[:, f, :],
            start=True, stop=True,
        )
        nc.vector.tensor_copy(out=T_sb[:, f, :], in_=pt[:])

    # per-node layer outputs (unpadded; row layout matches bounce rows)
    h1pad = const.tile([P, W, HID], bf16)
    h2pad = const.tile([P, W, OUT], bf16)
    final = const.tile([P, W, OUT], f32)

    # ---- embedding: h1pad[p, w, :] = dis * sum_f onehot_f_w.T @ T_f
    for w in range(W):
        oh = oh_pool.tile([NCAT, NF, P], bf16, tag="oh")
        nc.sync.dma_start(out=oh[:], in_=onehot[:, w, :, :])
        pe = psum_mm.tile([P, HID], f32, space="PSUM", tag="ps")
        for f in range(NF):
            nc.tensor.matmul(
                out=pe[:], lhsT=oh[:, f, :], rhs=T_sb[:, f, :],
                start=(f == 0), stop=(f == NF - 1),
            )
        nc.scalar.activation(out=h1pad[:, w, :], in_=pe[:], func=AF.Copy,
                             scale=dis_sb[:, w:w + 1])
        if (w + 1) % CH == 0:
            c0 = w + 1 - CH
            c = c0 // AGW
            r0 = c0 - c * AGW
            nc.sync.dma_start(out=bounce1[c][:, r0:r0 + CH, :],
                              in_=h1pad[:, c0:c0 + CH, :])
        if (w + 1) % AGW == 0:
            ag_chunk((w + 1) // AGW - 1, bounce1, agt1, table1_v, HID)

    def edge_layer(table, fdim, epilogue):
        """Gather+segment-sum over all edges; call epilogue(w, psum_tile)."""
        for g in range(NG):
            msgs = []
            for q in range(NQ):
                m = msg_pool.tile([P, GW * KQ, ROW], bf16, tag="msg")
                c0 = (q * W + g * GW) * KQ          # first column of chunk
                nc.gpsimd.dma_gather(
                    m[:], table[BUCK * q:BUCK * (q + 1), :],
                    idx_sb[:, c0 * 8:(c0 + GW * KQ) * 8],
                    num_idxs=GW * KQ * P, num_idxs_reg=GW * KQ * P,
                    elem_size=ROW, single_packet=False, queue_num=q,
                )
                msgs.append(m)
            for wi in range(GW):
                w = g * GW + wi
                s = s_pool.tile([P, NQ * KQ, P], bf16, tag="s")
                # S[p, (q,c), j] = (dstrel[p, col(q,w,c)] == j)
                nc.vector.tensor_tensor(
                    out=s.rearrange("p (q c) j -> p q c j", q=NQ),
                    in0=iota_sb.rearrange("p (o1 o2 j) -> p o1 o2 j",
                                          o1=1, o2=1)
                        .to_broadcast([P, NQ, KQ, P]),
                    in1=dstrel_sb.rearrange("p (q w c) -> p q w c", q=NQ, w=W)
                        [:, :, w, :]
                        .rearrange("p q (c o) -> p q c o", o=1)
                        .to_broadcast([P, NQ, KQ, P]),
                    op=ALU.is_equal,
                )
                pt = psum_mm.tile([P, fdim], f32, space="PSUM", tag="ps")
                k = 0
                for q in range(NQ):
                    for c in range(KQ):
                        nc.tensor.matmul(
                            out=pt[:],
                            lhsT=s[:, q * KQ + c, :],
                            rhs=msgs[q][:, wi * KQ + c, :fdim],
                            start=(k == 0), stop=(k == NQ * KQ - 1),
                        )
                        k += 1
                epilogue(w, pt)

    def normalize(x, mv_tag, out_ap, out_dtype_bf16):
        """x [P,F] fp32 -> out_ap = (x - mu) * rstd  (Scalar writes out_ap)."""
        stats = epi_pool.tile([P, 1, 6], f32, tag=mv_tag + "st")
        mv = epi_pool.tile([P, 2], f32, tag=mv_tag + "mv")
        nc.vector.bn_stats(out=stats[:, 0, :], in_=x[:])
        nc.vector.bn_aggr(out=mv[:], in_=stats[:])
        sd = epi_pool.tile([P, 1], f32, tag=mv_tag + "sd")
        nc.scalar.activation(out=sd[:], in_=mv[:, 1:2], func=AF.Sqrt,
                             bias=eps_sb[:], scale=1.0)
        rstd = epi_pool.tile([P, 1], f32, tag=mv_tag + "rs")
        nc.vector.reciprocal(out=rstd[:], in_=sd[:])
        negmu = epi_pool.tile([P, 1], f32, tag=mv_tag + "nm")
        nc.scalar.activation(out=negmu[:], in_=mv[:, 0:1], func=AF.Copy,
                             scale=-1.0)
        nmr = epi_pool.tile([P, 1], f32, tag=mv_tag + "nr")
        nc.vector.tensor_tensor(out=nmr[:], in0=negmu[:], in1=rstd[:],
                                op=ALU.mult)
        nc.scalar.activation(out=out_ap, in_=x[:], func=AF.Identity,
                             scale=rstd[:], bias=nmr[:])

    def epi1(w, pt):
        x = epi_pool.tile([P, HID], f32, tag="x1")
        # out1 = relu(dis*(psum + h1self) + b1)
        nc.vector.tensor_tensor(out=x[:], in0=pt[:], in1=h1pad[:, w, :],
                                op=ALU.add)
        nc.scalar.activation(out=x[:], in_=x[:], func=AF.Copy,
                             scale=dis_sb[:, w:w + 1])
        nc.vector.tensor_tensor(out=x[:], in0=x[:], in1=b1_sb[:], op=ALU.add)
        nc.scalar.activation(out=x[:], in_=x[:], func=AF.Relu)
        # gamma1/beta1 are folded into w2/c2 on the host, so only the
        # normalized value is needed downstream.
        y = epi_pool.tile([P, HID], bf16, tag="y1")
        normalize(x, "l1", y[:], True)
        # h2 = dis * (y @ W2g + c2): transpose y then matmul
        ptr = psum_tr.tile([HID, P], bf16, space="PSUM", tag="tr")
        nc.tensor.transpose(out=ptr[:], in_=y[:], identity=ident_sb[:])
        xT = epi_pool.tile([HID, P], bf16, tag="xT")
        nc.scalar.activation(out=xT[:], in_=ptr[:], func=AF.Copy)
        pw2 = psum_w2.tile([P, OUT], f32, space="PSUM", tag="w2")
        nc.tensor.matmul(out=pw2[:], lhsT=xT[:], rhs=w2_sb[:],
                         start=True, stop=True)
        h2t = epi_pool.tile([P, OUT], f32, tag="h2t")
        nc.vector.tensor_tensor(out=h2t[:], in0=pw2[:], in1=c2_sb[:],
                                op=ALU.add)
        nc.scalar.activation(out=h2pad[:, w, :], in_=h2t[:], func=AF.Copy,
                             scale=dis_sb[:, w:w + 1])
        if (w + 1) % CH == 0:
            c0 = w + 1 - CH
            c = c0 // AGW
            r0 = c0 - c * AGW
            nc.sync.dma_start(out=bounce2[c][:, r0:r0 + CH, :],
                              in_=h2pad[:, c0:c0 + CH, :])
        if (w + 1) % AGW == 0:
            ag_chunk((w + 1) // AGW - 1, bounce2, agt2, table2_v, OUT)

    def epi2(w, pt):
        x = epi_pool.tile([P, OUT], f32, tag="x2")
        nc.vector.tensor_tensor(out=x[:], in0=pt[:], in1=h2pad[:, w, :],
                                op=ALU.add)
        nc.scalar.activation(out=x[:], in_=x[:], func=AF.Copy,
                             scale=dis_sb[:, w:w + 1])
        nc.vector.tensor_tensor(out=x[:], in0=x[:], in1=b2_sb[:], op=ALU.add)
        xn = epi_pool.tile([P, OUT], f32, tag="xn2")
        normalize(x, "l2", xn[:], False)
        yg = epi_pool.tile([P, OUT], f32, tag="yg2")
        nc.vector.tensor_tensor(out=yg[:], in0=xn[:], in1=g2_sb[:],
                                op=ALU.mult)
        nc.vector.tensor_tensor(out=final[:, w, :], in0=yg[:], in1=be2_sb[:],
                                op=ALU.add)

    # ---- layer 1
    edge_layer(table1, HID, epi1)

    # ---- layer 2
    edge_layer(table2, OUT, epi2)

    nc.sync.dma_start(
        out=outx.rearrange("(p w) o -> p w o", p=P), in_=final[:])
    ctx.close()


# ------------------------------------------------------------------ host prep
def _pack_core(dloc, q_of_edge):
    """Assign local nodes to (window, slot) with per-(w,q) capacity CAP_Q and
    <=P nodes per window.  Returns win[SH], pslot[SH]."""
    # per-node per-bucket edge counts
    cnt = np.zeros((SH, NQ), np.int64)
    np.add.at(cnt, (dloc, q_of_edge), 1)
    tot = cnt.sum(1)
    order = np.argsort(-tot, kind="stable")
    fills = np.zeros((W, NQ), np.int64)
    counts = np.zeros(W, np.int64)
    win = np.zeros(SH, np.int64)
    for n in order:
        c = cnt[n]
        ok = (counts < P) & np.all(fills + c <= CAP_Q, axis=1)
        if not ok.any():
            raise RuntimeError("window packing failed")
        load = np.where(ok[:, None], fills + c, 1 << 30).max(axis=1)
        wsel = int(np.argmin(load))
        win[n] = wsel
        fills[wsel] += c
        counts[wsel] += 1
    # slot within window: order nodes by window
    pslot = np.zeros(SH, np.int64)
    for wsel in range(W):
        nodes = np.nonzero(win == wsel)[0]
        pslot[nodes] = np.arange(len(nodes))
    return win, pslot


def _to_bf16(a):
    return np.asarray(a, np.float32).astype(ml_dtypes.bfloat16)


def _host_prep(x_cat, edge_index, emb_tables, W1, b1, W2, b2,
               gamma1, beta1, gamma2, beta2):
    src = np.asarray(edge_index[0], np.int64)
    dst = np.asarray(edge_index[1], np.int64)
    deg = np.bincount(dst, minlength=N).astype(np.float64) + 1.0

    core_of = np.arange(N) // SH
    # pass 1: pack every core's nodes
    wins = np.zeros(N, np.int64)
    pslots = np.zeros(N, np.int64)
    srcq = src // (2 * SH)  # bucket of an edge = pair-of-cores owning src
    for k in range(NCORE):
        m = (dst // SH) == k
        dloc = dst[m] - k * SH
        win, ps = _pack_core(dloc, srcq[m])
        wins[k * SH:(k + 1) * SH] = win
        pslots[k * SH:(k + 1) * SH] = ps
    slot_of = pslots * W + wins               # p-major slot within owner core
    trow = core_of * SLOTS + slot_of          # global table row

    in_maps = []
    perm_slots = []
    for k in range(NCORE):
        m = (dst // SH) == k
        es, ed = src[m], dst[m] - k * SH
        ew = wins[ed + k * SH]
        ep = pslots[ed + k * SH]
        eq = trow[es] // BUCK
        # stream position: per (q, w) block of CAP_Q slots, fill in order
        gkey = eq * W + ew
        order = np.argsort(gkey, kind="stable")
        gsort = gkey[order]
        # rank within group
        start = np.searchsorted(gsort, np.arange(NQ * W))
        rank = np.arange(len(gsort)) - start[gsort]
        assert (rank < CAP_Q).all()
        pos = gsort * CAP_Q + rank
        idx16 = np.zeros(TOTPOS, np.int16)
        drel = np.full(TOTPOS, -1.0, np.float32)
        idx16[pos] = (trow[es][order] - eq[order] * BUCK).astype(np.int16)
        drel[pos] = ep[order].astype(np.float32)
        # wrap idx: j -> [j%16, j//16], replicate x8 partition groups
        idxw = np.tile(idx16.reshape(-1, 16).T, (8, 1))
        drelw = np.ascontiguousarray(drel.reshape(-1, P).T)

        # onehot [NCAT, W, NF, P] for this core's slots (bf16)
        oh = np.zeros((NCAT, W, NF, P), ml_dtypes.bfloat16)
        xc = np.asarray(x_cat[k * SH:(k + 1) * SH], np.int64)
        wloc = wins[k * SH:(k + 1) * SH]
        ploc = pslots[k * SH:(k + 1) * SH]
        for f in range(NF):
            oh[xc[:, f], wloc, f, ploc] = 1.0

        degs = np.ones((P, W), np.float32)
        degs[ploc, wloc] = deg[k * SH:(k + 1) * SH]

        embT = np.ascontiguousarray(
            np.asarray(emb_tables, np.float32).transpose(2, 0, 1)
            .reshape(EMB, NF * NCAT))

        rep = lambda v, d: np.broadcast_to(
            np.asarray(v, np.float32).reshape(1, d), (P, d)).copy()

        w2g = np.asarray(gamma1, np.float32)[:, None] * np.asarray(W2, np.float32)
        c2 = np.asarray(beta1, np.float32) @ np.asarray(W2, np.float32)

        in_maps.append({
            "onehot": oh,
            "idxs": idxw,
            "dstrel": _to_bf16(drelw),
            "deg": degs,
            "embT": embT,
            "w1": np.ascontiguousarray(np.asarray(W1, np.float32).reshape(NF, EMB, HID).transpose(1, 0, 2)),
            "w2": _to_bf16(w2g),
            "c2r": rep(c2, OUT),
            "b1r": rep(b1, HID),
            "b2r": rep(b2, OUT), "g2r": rep(gamma2, OUT),
            "be2r": rep(beta2, OUT),
            "iota": _to_bf16(np.broadcast_to(np.arange(P, dtype=np.float32), (P, P))),
            "ident": _to_bf16(np.eye(P, dtype=np.float32)),
        })
        perm_slots.append(slot_of[k * SH:(k + 1) * SH])
    return in_maps, perm_slots


# ------------------------------------------------------------------ entry
def kernel(x_cat, edge_index, emb_tables, W1, b1, W2, b2,
           gamma1, beta1, gamma2, beta2, _res_hook=None):
    if "nc" not in _CACHE:
        _CACHE["nc"] = build_program()
    nc = _CACHE["nc"]
    in_maps, perm_slots = _host_prep(
        np.asarray(x_cat), np.asarray(edge_index), np.asarray(emb_tables),
        np.asarray(W1), np.asarray(b1), np.asarray(W2), np.asarray(b2),
        np.asarray(gamma1), np.asarray(beta1), np.asarray(gamma2),
        np.asarray(beta2))
    res = run_bass_kernel_spmd(nc, in_maps, list(range(NCORE)),
                               **(_res_hook or {}))
    out = np.empty((N, OUT), np.float32)
    for k in range(NCORE):
        full = res.results[k]["outx"]        # [SLOTS, OUT] slot-ordered
        out[k * SH:(k + 1) * SH] = full[perm_slots[k]]
    if _res_hook is not None:
        _res_hook["result"] = res
    return out


# revision 33
# speedup vs baseline: 1.0433x; 1.0433x over previous
"""CategoricalGCNEncoder on 8 Trainium2 NeuronCores (Bass/Tile).

Design ("v6" — bf16 + engine rebalance over v5):
  - Nodes (dst) sharded across 8 cores; per-layer feature tables AllGathered.
  - Everything on the matmul path is bf16 (fp32 matmuls cost 4x stream +
    2x LDWEIGHTS); PSUM accumulation stays fp32, epilogue math fp32.
  - Slot numbering is p-major (slot = p*W + w) so bounce DMAs are
    per-partition contiguous, issued in window chunks and overlapped with
    compute instead of serializing before each AllGather.
  - Embedding + first matmul fused: h1 = sum_f onehot_f @ T_f with
    T_f = emb_f @ W1_f computed on device; one-hot is uploaded bf16 in a
    window-contiguous layout [NCAT, W, NF, P].
  - GCN normalization folded into node scaling: table rows hold dis*h;
    out[d] = dis[d] * (sum_{e:dst=d} ht[src] + ht[d]) + b.
  - Edge phase: per-edge rows gathered with gpsimd.dma_gather (int16 idx,
    256B bf16-padded rows) from the AllGathered table, 4 SWDGE queues =
    4 src "buckets"; gathers batched 8 windows at a time.
  - Segment-sum: per dst-window (128 nodes) PSUM accumulation of
    matmul(lhsT=S_col[128x128], rhs=msg_col[128xF]) where S is the one-hot
    dst-slot selection matrix generated on DVE via is_equal(iota, dstrel),
    all bf16. Pad edges carry dstrel=-1 -> zero S column.
  - Epilogue per-partition scale/bias ops run on the (otherwise idle)
    Scalar engine via activation(scale=AP,bias=AP); DVE keeps only true
    tensor-tensor work.
  - Host packs nodes into windows (vector bin packing) so every (window,
    bucket) has exactly 4 columns of 128 edge slots; the node->slot
    permutation is undone on the host at the end.
"""

import numpy as np
import ml_dtypes

import concourse.bass as bass
import concourse.mybir as mybir
import concourse.tile as tile
from concourse import bacc
from concourse.bass_utils import run_bass_kernel_spmd

# ---------------- problem constants (hardcoded; kernel must be self-contained)
N = 100000
E = 1600000
NF = 8
EMB = 16
IN_DIM = 128
HID = 64
OUT = 32
NCAT = 100
EPS = 1e-5

NCORE = 8
SH = N // NCORE            # 12500 nodes per core
P = 128
W = 104                    # windows per core
SLOTS = W * P              # 13312 slots per core (>= SH)
KQ = 4                     # columns per (window, bucket)
NQ = 4                     # src buckets == SWDGE queues
COLS = W * KQ              # columns per bucket stream (416)
TOTCOL = NQ * COLS         # total columns (1664)
TOTPOS = TOTCOL * P        # total edge slots (212992)
TBL = NCORE * SLOTS        # table rows (106496)
BUCK = TBL // NQ           # bucket size (26624) < 32768
GW = 8                     # windows per gather group
NG = W // GW               # gather groups (13)
CAP_Q = KQ * P             # 512 edge slots per (w, q)
ROW = 128                  # bf16 elems per table row (= 256B, SWDGE minimum)
CH = 13                    # bounce-DMA chunk (windows)
AGW = 26                   # windows per chunked AllGather (4 chunks)
NAG = W // AGW             # AllGather chunks == src buckets (4)
# bucket q of an edge = window-chunk of its src node; the per-chunk
# AllGather output tensors ARE the gather tables (no repad pass).

f32 = mybir.dt.float32
bf16 = mybir.dt.bfloat16
i16 = mybir.dt.int16

_CACHE = {}


# ------------------------------------------------------------------ program
def build_program():
    nc = bacc.Bacc(None, target_bir_lowering=False, debug=False,
                   num_devices=NCORE, num_swdge_queues=NQ,
                   dynamic_dma_scratch_size=16384)
    with tile.TileContext(nc) as tc:
        _build(nc, tc)
    nc.compile()
    return nc


def _build(nc, tc):
    AF = mybir.ActivationFunctionType
    ALU = mybir.AluOpType

    from contextlib import ExitStack
    ctx = ExitStack()
    dram = ctx.enter_context(tc.tile_pool(name="dram", bufs=1, space="DRAM"))
    const = ctx.enter_context(tc.tile_pool(name="const", bufs=1))
    oh_pool = ctx.enter_context(tc.tile_pool(name="ohp", bufs=3))
    msg_pool = ctx.enter_context(tc.tile_pool(name="msgp", bufs=6))
    s_pool = ctx.enter_context(tc.tile_pool(name="sp", bufs=3))
    epi_pool = ctx.enter_context(tc.tile_pool(name="epip", bufs=3))
    psum_mm = ctx.enter_context(tc.tile_pool(name="psmm", bufs=3, space="PSUM"))
    psum_tr = ctx.enter_context(tc.tile_pool(name="pstr", bufs=2, space="PSUM"))
    psum_w2 = ctx.enter_context(tc.tile_pool(name="psw2", bufs=2, space="PSUM"))

    def din(name, shape, dtype=f32):
        return dram.tile(shape, dtype, kind="ExternalInput", name=name,
                         uniquify=False)

    # ---- inputs
    onehot = din("onehot", [NCAT, W, NF, P], bf16)
    idxs = din("idxs", [P, TOTPOS // 16], i16)
    dstrel = din("dstrel", [P, TOTCOL], bf16)
    degin = din("deg", [P, W])
    embT = din("embT", [EMB, NF * NCAT])
    w1 = din("w1", [EMB, NF, HID])
    w2 = din("w2", [HID, OUT], bf16)     # pre-scaled by gamma1 on host
    c2r = din("c2r", [P, OUT])           # beta1 @ W2, replicated
    b1r = din("b1r", [P, HID])
    b2r = din("b2r", [P, OUT])
    g2r = din("g2r", [P, OUT])
    be2r = din("be2r", [P, OUT])
    iotain = din("iota", [P, P], bf16)
    identin = din("ident", [P, P], bf16)

    outx = dram.tile([SLOTS, OUT], f32, kind="ExternalOutput", name="outx",
                     uniquify=False)

    # per-chunk contiguous padded bounce tensors (collective inputs must be
    # contiguous): row (p*AGW + wrel) <-> h1pad[:, c*AGW + wrel, :].
    # The AllGather outputs agt*_c are the gather tables: bucket q rows are
    # (k, p, wrel) of chunk q; 256B rows = [payload, zero-pad].
    bounce1 = [dram.tile([P, AGW, ROW], bf16, name=f"bounce1_{c}")
               for c in range(NAG)]
    bounce2 = [dram.tile([P, AGW, ROW], bf16, name=f"bounce2_{c}")
               for c in range(NAG)]
    agt1 = [dram.tile([NCORE, P, AGW, ROW], bf16, addr_space="Shared",
                      name=f"agt1_{c}") for c in range(NAG)]
    agt2 = [dram.tile([NCORE, P, AGW, ROW], bf16, addr_space="Shared",
                      name=f"agt2_{c}") for c in range(NAG)]

    def ag_chunk(c, bounce, agt):
        nc.gpsimd.collective_compute(
            "AllGather", mybir.AluOpType.bypass,
            replica_groups=[list(range(NCORE))],
            ins=[bounce[c][:]], outs=[agt[c][:]],
        )

    # ---- static SBUF
    idx_sb = const.tile([P, TOTPOS // 16], i16)
    nc.sync.dma_start(out=idx_sb[:], in_=idxs[:])
    dstrel_sb = const.tile([P, TOTCOL], bf16)
    nc.sync.dma_start(out=dstrel_sb[:], in_=dstrel[:])
    iota_sb = const.tile([P, P], bf16)
    nc.sync.dma_start(out=iota_sb[:], in_=iotain[:])
    ident_sb = const.tile([P, P], bf16)
    nc.sync.dma_start(out=ident_sb[:], in_=identin[:])
    w1_sb = const.tile([EMB, NF, HID], f32)
    nc.sync.dma_start(out=w1_sb[:], in_=w1[:])
    w2_sb = const.tile([HID, OUT], bf16)
    nc.sync.dma_start(out=w2_sb[:], in_=w2[:])
    embT_sb = const.tile([EMB, NF * NCAT], f32)
    nc.sync.dma_start(out=embT_sb[:], in_=embT[:])
    b1_sb = const.tile([P, HID], f32)
    nc.sync.dma_start(out=b1_sb[:], in_=b1r[:])
    c2_sb = const.tile([P, OUT], f32)
    nc.sync.dma_start(out=c2_sb[:], in_=c2r[:])
    b2_sb = const.tile([P, OUT], f32)
    nc.sync.dma_start(out=b2_sb[:], in_=b2r[:])
    g2_sb = const.tile([P, OUT], f32)
    nc.sync.dma_start(out=g2_sb[:], in_=g2r[:])
    be2_sb = const.tile([P, OUT], f32)
    nc.sync.dma_start(out=be2_sb[:], in_=be2r[:])
    eps_sb = const.tile([P, 1], f32)
    nc.vector.memset(eps_sb[:], EPS)

    # dis = 1/sqrt(deg)
    deg_sb = const.tile([P, W], f32)
    nc.sync.dma_start(out=deg_sb[:], in_=degin[:])
    dis_sb = const.tile([P, W], f32)
    nc.scalar.activation(out=dis_sb[:], in_=deg_sb[:], func=AF.Sqrt)
    nc.vector.reciprocal(out=dis_sb[:], in_=dis_sb[:])

    # ---- T_f = emb_f @ W1_f  -> T_sb [NCAT, NF, HID] bf16
    T_sb = const.tile([NCAT, NF, HID], bf16)
    for f in range(NF):
        pt = psum_mm.tile([NCAT, HID], f32, space="PSUM", tag="ps")
        nc.tensor.matmul(
            out=pt[:],
            lhsT=embT_sb[:, f * NCAT:(f + 1) * NCAT],
            rhs=w1_sb[:, f, :],
            start=True, stop=True,
        )
        nc.vector.tensor_copy(out=T_sb[:, f, :], in_=pt[:])

    # padded per-node layer outputs (row layout matches bounce rows)
    h1pad = const.tile([P, W, ROW], bf16)
    nc.vector.memset(h1pad[:], 0.0)
    h2pad = const.tile([P, W, ROW], bf16)
    nc.vector.memset(h2pad[:], 0.0)
    final = const.tile([P, W, OUT], f32)

    # ---- embedding: h1pad[p, w, :] = dis * sum_f onehot_f_w.T @ T_f
    for w in range(W):
        oh = oh_pool.tile([NCAT, NF, P], bf16, tag="oh")
        nc.sync.dma_start(out=oh[:], in_=onehot[:, w, :, :])
        pe = psum_mm.tile([P, HID], f32, space="PSUM", tag="ps")
        for f in range(NF):
            nc.tensor.matmul(
                out=pe[:], lhsT=oh[:, f, :], rhs=T_sb[:, f, :],
                start=(f == 0), stop=(f == NF - 1),
            )
        nc.scalar.activation(out=h1pad[:, w, :HID], in_=pe[:], func=AF.Copy,
                             scale=dis_sb[:, w:w + 1])
        if (w + 1) % CH == 0:
            c0 = w + 1 - CH
            c = c0 // AGW
            r0 = c0 - c * AGW
            nc.sync.dma_start(out=bounce1[c][:, r0:r0 + CH, :],
                              in_=h1pad[:, c0:c0 + CH, :])
        if (w + 1) % AGW == 0:
            ag_chunk((w + 1) // AGW - 1, bounce1, agt1)

    def edge_layer(agt, fdim, epilogue):
        """Gather+segment-sum over all edges; call epilogue(w, psum_tile)."""
        tables = [t.rearrange("k p w h -> (k p w) h") for t in agt]
        for g in range(NG):
            msgs = []
            for q in range(NQ):
                m = msg_pool.tile([P, GW * KQ, ROW], bf16, tag="msg")
                c0 = (q * W + g * GW) * KQ          # first column of chunk
                nc.gpsimd.dma_gather(
                    m[:], tables[q][:],
                    idx_sb[:, c0 * 8:(c0 + GW * KQ) * 8],
                    num_idxs=GW * KQ * P, num_idxs_reg=GW * KQ * P,
                    elem_size=ROW, single_packet=False, queue_num=q,
                )
                msgs.append(m)
            for wi in range(GW):
                w = g * GW + wi
                s = s_pool.tile([P, NQ * KQ, P], bf16, tag="s")
                # S[p, (q,c), j] = (dstrel[p, col(q,w,c)] == j)
                nc.vector.tensor_tensor(
                    out=s.rearrange("p (q c) j -> p q c j", q=NQ),
                    in0=iota_sb.rearrange("p (o1 o2 j) -> p o1 o2 j",
                                          o1=1, o2=1)
                        .to_broadcast([P, NQ, KQ, P]),
                    in1=dstrel_sb.rearrange("p (q w c) -> p q w c", q=NQ, w=W)
                        [:, :, w, :]
                        .rearrange("p q (c o) -> p q c o", o=1)
                        .to_broadcast([P, NQ, KQ, P]),
                    op=ALU.is_equal,
                )
                pt = psum_mm.tile([P, fdim], f32, space="PSUM", tag="ps")
                k = 0
                for q in range(NQ):
                    for c in range(KQ):
                        nc.tensor.matmul(
                            out=pt[:],
                            lhsT=s[:, q * KQ + c, :],
                            rhs=msgs[q][:, wi * KQ + c, :fdim],
                            start=(k == 0), stop=(k == NQ * KQ - 1),
                        )
                        k += 1
                epilogue(w, pt)

    def normalize(x, mv_tag, out_ap, out_dtype_bf16):
        """x [P,F] fp32 -> out_ap = (x - mu) * rstd  (Scalar writes out_ap)."""
        stats = epi_pool.tile([P, 1, 6], f32, tag=mv_tag + "st")
        mv = epi_pool.tile([P, 2], f32, tag=mv_tag + "mv")
        nc.vector.bn_stats(out=stats[:, 0, :], in_=x[:])
        nc.vector.bn_aggr(out=mv[:], in_=stats[:])
        sd = epi_pool.tile([P, 1], f32, tag=mv_tag + "sd")
        nc.scalar.activation(out=sd[:], in_=mv[:, 1:2], func=AF.Sqrt,
                             bias=eps_sb[:], scale=1.0)
        rstd = epi_pool.tile([P, 1], f32, tag=mv_tag + "rs")
        nc.vector.reciprocal(out=rstd[:], in_=sd[:])
        negmu = epi_pool.tile([P, 1], f32, tag=mv_tag + "nm")
        nc.scalar.activation(out=negmu[:], in_=mv[:, 0:1], func=AF.Copy,
                             scale=-1.0)
        nmr = epi_pool.tile([P, 1], f32, tag=mv_tag + "nr")
        nc.vector.tensor_tensor(out=nmr[:], in0=negmu[:], in1=rstd[:],
                                op=ALU.mult)
        nc.scalar.activation(out=out_ap, in_=x[:], func=AF.Identity,
                             scale=rstd[:], bias=nmr[:])

    def epi1(w, pt):
        x = epi_pool.tile([P, HID], f32, tag="x1")
        # out1 = relu(dis*(psum + h1self) + b1)
        nc.vector.tensor_tensor(out=x[:], in0=pt[:], in1=h1pad[:, w, :HID],
                                op=ALU.add)
        nc.scalar.activation(out=x[:], in_=x[:], func=AF.Copy,
                             scale=dis_sb[:, w:w + 1])
        nc.vector.tensor_tensor(out=x[:], in0=x[:], in1=b1_sb[:], op=ALU.add)
        nc.scalar.activation(out=x[:], in_=x[:], func=AF.Relu)
        # gamma1/beta1 are folded into w2/c2 on the host, so only the
        # normalized value is needed downstream.
        y = epi_pool.tile([P, HID], bf16, tag="y1")
        normalize(x, "l1", y[:], True)
        # h2 = dis * (y @ W2g + c2): transpose y then matmul
        ptr = psum_tr.tile([HID, P], bf16, space="PSUM", tag="tr")
        nc.tensor.transpose(out=ptr[:], in_=y[:], identity=ident_sb[:])
        xT = epi_pool.tile([HID, P], bf16, tag="xT")
        nc.scalar.activation(out=xT[:], in_=ptr[:], func=AF.Copy)
        pw2 = psum_w2.tile([P, OUT], f32, space="PSUM", tag="w2")
        nc.tensor.matmul(out=pw2[:], lhsT=xT[:], rhs=w2_sb[:],
                         start=True, stop=True)
        h2t = epi_pool.tile([P, OUT], f32, tag="h2t")
        nc.vector.tensor_tensor(out=h2t[:], in0=pw2[:], in1=c2_sb[:],
                                op=ALU.add)
        nc.scalar.activation(out=h2pad[:, w, :OUT], in_=h2t[:], func=AF.Copy,
                             scale=dis_sb[:, w:w + 1])
        if (w + 1) % CH == 0:
            c0 = w + 1 - CH
            c = c0 // AGW
            r0 = c0 - c * AGW
            nc.sync.dma_start(out=bounce2[c][:, r0:r0 + CH, :],
                              in_=h2pad[:, c0:c0 + CH, :])
        if (w + 1) % AGW == 0:
            ag_chunk((w + 1) // AGW - 1, bounce2, agt2)

    def epi2(w, pt):
        x = epi_pool.tile([P, OUT], f32, tag="x2")
        nc.vector.tensor_tensor(out=x[:], in0=pt[:], in1=h2pad[:, w, :OUT],
                                op=ALU.add)
        nc.scalar.activation(out=x[:], in_=x[:], func=AF.Copy,
                             scale=dis_sb[:, w:w + 1])
        nc.vector.tensor_tensor(out=x[:], in0=x[:], in1=b2_sb[:], op=ALU.add)
        xn = epi_pool.tile([P, OUT], f32, tag="xn2")
        normalize(x, "l2", xn[:], False)
        yg = epi_pool.tile([P, OUT], f32, tag="yg2")
        nc.vector.tensor_tensor(out=yg[:], in0=xn[:], in1=g2_sb[:],
                                op=ALU.mult)
        nc.vector.tensor_tensor(out=final[:, w, :], in0=yg[:], in1=be2_sb[:],
                                op=ALU.add)

    # ---- layer 1
    edge_layer(agt1, HID, epi1)

    # ---- layer 2
    edge_layer(agt2, OUT, epi2)

    nc.sync.dma_start(
        out=outx.rearrange("(p w) o -> p w o", p=P), in_=final[:])
    ctx.close()


# ------------------------------------------------------------------ host prep
def _pack_nodes(nloc, q_of_edge, dloc, nwin):
    """Assign `nloc` nodes to (window, slot) with per-(w,q) capacity CAP_Q
    and <=P nodes per window.  Returns win[nloc], pslot[nloc]."""
    # per-node per-bucket edge counts
    cnt = np.zeros((nloc, NQ), np.int64)
    np.add.at(cnt, (dloc, q_of_edge), 1)
    tot = cnt.sum(1)
    order = np.argsort(-tot, kind="stable")
    fills = np.zeros((nwin, NQ), np.int64)
    counts = np.zeros(nwin, np.int64)
    win = np.zeros(nloc, np.int64)
    for n in order:
        c = cnt[n]
        ok = (counts < P) & np.all(fills + c <= CAP_Q, axis=1)
        if not ok.any():
            raise RuntimeError("window packing failed")
        load = np.where(ok[:, None], fills + c, 1 << 30).max(axis=1)
        wsel = int(np.argmin(load))
        win[n] = wsel
        fills[wsel] += c
        counts[wsel] += 1
    # slot within window: order nodes by window
    pslot = np.zeros(nloc, np.int64)
    for wsel in range(nwin):
        nodes = np.nonzero(win == wsel)[0]
        pslot[nodes] = np.arange(len(nodes))
    return win, pslot


def _to_bf16(a):
    return np.asarray(a, np.float32).astype(ml_dtypes.bfloat16)


def _host_prep(x_cat, edge_index, emb_tables, W1, b1, W2, b2,
               gamma1, beta1, gamma2, beta2):
    src = np.asarray(edge_index[0], np.int64)
    dst = np.asarray(edge_index[1], np.int64)
    deg = np.bincount(dst, minlength=N).astype(np.float64) + 1.0

    core_of = np.arange(N) // SH
    # bucket (= AllGather chunk) of a node: fixed upfront so packing of one
    # core doesn't depend on another core's packing
    chunk_of = (np.arange(N) % SH) % NAG
    # pass 1: pack every (core, chunk)'s nodes into its AGW windows
    wins = np.zeros(N, np.int64)
    pslots = np.zeros(N, np.int64)
    srcq = chunk_of[src]  # bucket of an edge = src node's chunk
    for k in range(NCORE):
        for c in range(NAG):
            m = np.nonzero((core_of == k) & (chunk_of == c))[0]  # node ids
            me = (dst // SH == k) & (chunk_of[dst] == c)         # edge mask
            # local index of dst within this (core, chunk) group
            loc = np.full(N, -1, np.int64)
            loc[m] = np.arange(len(m))
            win, ps = _pack_nodes(len(m), srcq[me], loc[dst[me]], AGW)
            wins[m] = c * AGW + win
            pslots[m] = ps
    slot_of = pslots * W + wins               # p-major slot within owner core
    # row within the bucket (chunk tensor): (k, p, wrel)
    rowq = core_of * (P * AGW) + pslots * AGW + (wins - chunk_of * AGW)

    in_maps = []
    perm_slots = []
    for k in range(NCORE):
        m = (dst // SH) == k
        es, ed = src[m], dst[m] - k * SH
        ew = wins[ed + k * SH]
        ep = pslots[ed + k * SH]
        eq = chunk_of[es]
        # stream position: per (q, w) block of CAP_Q slots, fill in order
        gkey = eq * W + ew
        order = np.argsort(gkey, kind="stable")
        gsort = gkey[order]
        # rank within group
        start = np.searchsorted(gsort, np.arange(NQ * W))
        rank = np.arange(len(gsort)) - start[gsort]
        assert (rank < CAP_Q).all()
        pos = gsort * CAP_Q + rank
        idx16 = np.zeros(TOTPOS, np.int16)
        drel = np.full(TOTPOS, -1.0, np.float32)
        idx16[pos] = rowq[es][order].astype(np.int16)
        drel[pos] = ep[order].astype(np.float32)
        # wrap idx: j -> [j%16, j//16], replicate x8 partition groups
        idxw = np.tile(idx16.reshape(-1, 16).T, (8, 1))
        drelw = np.ascontiguousarray(drel.reshape(-1, P).T)

        # onehot [NCAT, W, NF, P] for this core's slots (bf16)
        oh = np.zeros((NCAT, W, NF, P), ml_dtypes.bfloat16)
        xc = np.asarray(x_cat[k * SH:(k + 1) * SH], np.int64)
        wloc = wins[k * SH:(k + 1) * SH]
        ploc = pslots[k * SH:(k + 1) * SH]
        for f in range(NF):
            oh[xc[:, f], wloc, f, ploc] = 1.0

        degs = np.ones((P, W), np.float32)
        degs[ploc, wloc] = deg[k * SH:(k + 1) * SH]

        embT = np.ascontiguousarray(
            np.asarray(emb_tables, np.float32).transpose(2, 0, 1)
            .reshape(EMB, NF * NCAT))

        rep = lambda v, d: np.broadcast_to(
            np.asarray(v, np.float32).reshape(1, d), (P, d)).copy()

        w2g = np.asarray(gamma1, np.float32)[:, None] * np.asarray(W2, np.float32)
        c2 = np.asarray(beta1, np.float32) @ np.asarray(W2, np.float32)

        in_maps.append({
            "onehot": oh,
            "idxs": idxw,
            "dstrel": _to_bf16(drelw),
            "deg": degs,
            "embT": embT,
            "w1": np.ascontiguousarray(np.asarray(W1, np.float32).reshape(NF, EMB, HID).transpose(1, 0, 2)),
            "w2": _to_bf16(w2g),
            "c2r": rep(c2, OUT),
            "b1r": rep(b1, HID),
            "b2r": rep(b2, OUT), "g2r": rep(gamma2, OUT),
            "be2r": rep(beta2, OUT),
            "iota": _to_bf16(np.broadcast_to(np.arange(P, dtype=np.float32), (P, P))),
            "ident": _to_bf16(np.eye(P, dtype=np.float32)),
        })
        perm_slots.append(slot_of[k * SH:(k + 1) * SH])
    return in_maps, perm_slots


# ------------------------------------------------------------------ entry
def kernel(x_cat, edge_index, emb_tables, W1, b1, W2, b2,
           gamma1, beta1, gamma2, beta2, _res_hook=None):
    if "nc" not in _CACHE:
        _CACHE["nc"] = build_program()
    nc = _CACHE["nc"]
    in_maps, perm_slots = _host_prep(
        np.asarray(x_cat), np.asarray(edge_index), np.asarray(emb_tables),
        np.asarray(W1), np.asarray(b1), np.asarray(W2), np.asarray(b2),
        np.asarray(gamma1), np.asarray(beta1), np.asarray(gamma2),
        np.asarray(beta2))
    res = run_bass_kernel_spmd(nc, in_maps, list(range(NCORE)),
                               **(_res_hook or {}))
    out = np.empty((N, OUT), np.float32)
    for k in range(NCORE):
        full = res.results[k]["outx"]        # [SLOTS, OUT] slot-ordered
        out[k * SH:(k + 1) * SH] = full[perm_slots[k]]
    if _res_hook is not None:
        _res_hook["result"] = res
    return out
